# revision 16
# baseline (speedup 1.0000x reference)
"""Cross-layer transcoder kernel for 8 TRN2 NeuronCores.

Sharding: d_transcoder (F=4096) is split 8 ways (512 features per core).
Each core encodes all tokens against its feature slice, computes partial
cross-layer reconstructions for every target layer, and a chunked
ReduceScatter sums the partials; rank i receives target layer i
(d-major [D, B] per core, L == n_cores == 8).  The host transposes each
rank's output and adds the decoder bias.

Decode dataflow: W_dec tiles are the stationary operand, feats stream
512-token windows -> psum [128 d, 512 tok] x 6 d-tiles (6 banks), all
streams 512 long so LDWEIGHTS always hides under the previous stream.

Compute dtype: bf16 operands with fp32 PSUM accumulation, except target
layers F8J which are decoded in fp8-e4m3 DoubleRow (2 contraction slots
per pass, 2x PE throughput).  fp8 operands are pre-scaled (feats x16 on
device via DVE, W_dec x256 on host) and the drain de-scales by 2^-12.

All HBM streams use host-pretransposed contiguous layouts (>=6KB per
partition line) so the DMA stays efficient while the collective runs.

Queue routing: weight/x streaming loads on the Sync queue; partial
stores on the Scalar queue (after the DVE drains); the collective and
post-RS output copies on the GpSimd queue.
"""

import numpy as np
import ml_dtypes

L, B, D, F = 8, 2048, 768, 4096
NCORES = 8
FL = F // NCORES          # 512 features per core
AF = FL // 128            # 4 f-tiles per core
AF2 = AF // 2             # 2 fp8 double-tiles per core
DT = D // 128              # 6 d-tiles
EH = 512                  # encode token chunk per x DMA
WINS = [512, 512, 512, 512]
assert sum(WINS) == B

F8J = (0, 6)              # target layers decoded in fp8 DoubleRow
SF = 16.0                 # feats fp8 scale
SW = 256.0                # W_dec fp8 scale
DESCALE = 1.0 / (SF * SW)

_COMPILED_NC = None


def _build_nc():
    import concourse.mybir as mybir
    import concourse.tile as tile
    from concourse import bacc

    dt = mybir.dt
    nc = bacc.Bacc("TRN2", target_bir_lowering=False, debug=False,
                   num_devices=NCORES)

    HN = B // EH
    xt = nc.dram_tensor("xt", [L, HN, 128, DT, EH], dt.bfloat16,
                        kind="ExternalInput").ap()
    wenc = nc.dram_tensor("wenc", [L, 128, DT, FL], dt.bfloat16,
                          kind="ExternalInput").ap()
    benc = nc.dram_tensor("benc", [128, L * AF], dt.float32,
                          kind="ExternalInput").ap()
    wdec = nc.dram_tensor("wdec", [L, L, 128, AF, D], dt.bfloat16,
                          kind="ExternalInput").ap()
    wdec8 = nc.dram_tensor("wdec8", [L, len(F8J), 128, AF2, 2, D],
                           dt.float8e4, kind="ExternalInput").ap()
    out = nc.dram_tensor("out", [D, B], dt.bfloat16, kind="ExternalOutput").ap()

    RELU = mybir.ActivationFunctionType.Relu
    DR = mybir.MatmulPerfMode.DoubleRow

    with tile.TileContext(nc) as tc:
        with (
            tc.tile_pool(name="consts", bufs=1) as consts,
            tc.tile_pool(name="featp", bufs=L * AF) as featp,
            tc.tile_pool(name="decp", bufs=8) as decp,
            tc.tile_pool(name="dec8p", bufs=2) as dec8p,
            tc.tile_pool(name="dram", bufs=1, space="DRAM") as dram,
        ):
            feats = [
                [featp.tile([128, B], dt.bfloat16, name=f"feat_{l}_{a}",
                            tag="feat", bufs=L * AF) for a in range(AF)]
                for l in range(L)
            ]

            rs_in = [dram.tile([L, D, wb], dt.bfloat16, name=f"rs_in_{w}",
                               tag=f"rsin{w}") for w, wb in enumerate(WINS)]
            rs_out = [dram.tile([D, wb], dt.bfloat16, name=f"rs_out_{w}",
                                tag=f"rsout{w}") for w, wb in enumerate(WINS)]

            def load_wd(w, j, l):
                t = decp.tile([128, AF, D], dt.bfloat16, tag="wd", bufs=8,
                              name=f"wd_{w}_{j}_{l}")
                nc.sync.dma_start(t[:], wdec[l, j])
                return t

            def load_wd8(w, j8, l):
                t = dec8p.tile([128, AF2, 2, D], dt.float8e4, tag="wd8",
                               bufs=2, name=f"wd8_{w}_{j8}_{l}")
                nc.sync.dma_start(t[:], wdec8[l, j8])
                return t

            prefetched = {}

            # ---- Phase E: encode all layers/tokens; feats stay in SBUF ----
            with (
                tc.tile_pool(name="encp", bufs=2) as encp,
                tc.tile_pool(name="pep", bufs=3, space="PSUM") as pep,
            ):
                benc_t = None
                for l in range(L):
                    wenc_t = encp.tile([128, DT, FL], dt.bfloat16,
                                       tag="wenc_t", bufs=2, name=f"wenc_{l}")
                    nc.sync.dma_start(wenc_t[:], wenc[l])
                    for h in range(HN):
                        xt_t = encp.tile([128, DT, EH], dt.bfloat16,
                                         tag="xt_t", bufs=2, name=f"xt_{l}_{h}")
                        nc.sync.dma_start(xt_t[:], xt[l, h])
                        if l == 0 and h == 0 and benc_t is None:
                            benc_t = consts.tile([128, L * AF], dt.float32,
                                                 tag="benc_t")
                            nc.sync.dma_start(benc_t[:], benc)
                        for a in range(AF):
                            ps = pep.tile([128, EH], dt.float32,
                                          tag="pe", bufs=3,
                                          name=f"pe_{l}_{h}_{a}")
                            for k in range(DT):
                                nc.tensor.matmul(
                                    ps[:],
                                    wenc_t[:, k, a * 128:(a + 1) * 128],
                                    xt_t[:, k, :],
                                    start=(k == 0), stop=(k == DT - 1))
                            boff = h * EH
                            nc.scalar.activation(
                                feats[l][a][:, boff:boff + EH], ps[:],
                                RELU,
                                bias=benc_t[:, l * AF + a:l * AF + a + 1])
                    if l == 1:
                        # prefetch first bf16 decode weights behind encode
                        prefetched[(0, 7, 0)] = load_wd(0, 7, 0)
                        prefetched[(0, 7, 1)] = load_wd(0, 7, 1)
                        prefetched[(0, 7, 2)] = load_wd(0, 7, 2)

            # ---- Phase D: cross-layer decode + chunked ReduceScatter ----
            with (
                tc.tile_pool(name="f8p",
                             bufs=(max(F8J) + 1) * AF2 if F8J else 1) as f8p,
                tc.tile_pool(name="outp", bufs=6) as outp,
                tc.tile_pool(name="pdp", bufs=8, space="PSUM") as pdp,
            ):
                def convert_win(w):
                    # fp8 feats for window w (DVE runs these while the PE
                    # is busy with the previous window's last layers)
                    wb = WINS[w]
                    boff = sum(WINS[:w])
                    lmax = max(F8J)
                    d = {}
                    for l in range(lmax + 1):
                        for t2 in range(AF2):
                            ft = f8p.tile([128, 2, 512], dt.float8e4,
                                          tag="f8", bufs=(lmax + 1) * AF2,
                                          name=f"f8_{w}_{l}_{t2}")
                            for i in range(2):
                                nc.scalar.activation(
                                    ft[:, i, 0:wb],
                                    feats[l][2 * t2 + i][:, boff:boff + wb],
                                    RELU, scale=SF)
                            d[(l, t2)] = ft
                    return d

                f8wins = [None] * len(WINS)
                if F8J:
                    f8wins[0] = convert_win(0)
                # j=7 first: its 8 wd tiles fill the whole prefetch pool
                # during the previous window, so the PE needs no new loads
                # while the ReduceScatter burst is hogging the DMA engines.
                J_ORDER = [7, 0, 1, 2, 3, 4, 5, 6]
                boff = 0
                for w, wb in enumerate(WINS):
                    f8win = f8wins[w]
                    for j in J_ORDER:
                        p6 = [pdp.tile([128, 512], dt.float32, tag="p6",
                                       bufs=8, name=f"p6_{w}_{j}_{q}")
                              for q in range(DT)]
                        if j in F8J:
                            j8 = F8J.index(j)
                            for l in range(j + 1):
                                wd8 = load_wd8(w, j8, l)
                                st = (l == 0)
                                sp = (l == j)
                                for t2 in range(AF2):
                                    for q in range(DT):
                                        nc.tensor.matmul(
                                            p6[q][:, 0:wb],
                                            wd8[:, t2, :,
                                                q * 128:(q + 1) * 128],
                                            f8win[(l, t2)][:, :, 0:wb],
                                            start=(st and t2 == 0),
                                            stop=(sp and t2 == AF2 - 1),
                                            perf_mode=DR)
                        else:
                            for l in range(j + 1):
                                wd = prefetched.pop((w, j, l), None)
                                if wd is None:
                                    wd = load_wd(w, j, l)
                                st = (l == 0)
                                sp = (l == j)
                                for a in range(AF):
                                    for q in range(DT):
                                        nc.tensor.matmul(
                                            p6[q][:, 0:wb],
                                            wd[:, a, q * 128:(q + 1) * 128],
                                            feats[l][a][:, boff:boff + wb],
                                            start=(st and a == 0),
                                            stop=(sp and a == AF - 1))
                        dscale = DESCALE if j in F8J else 1.0
                        for q in range(DT):
                            ot = outp.tile([128, 512], dt.bfloat16, tag="ot",
                                           bufs=6, name=f"ot_{w}_{j}_{q}")
                            nc.vector.tensor_scalar_mul(
                                ot[:, 0:wb], p6[q][:, 0:wb], dscale)
                            nc.scalar.dma_start(
                                rs_in[w][j, q * 128:(q + 1) * 128, :],
                                ot[:, 0:wb])
                        if (F8J and j == max(F8J)
                                and w + 1 < len(WINS)):
                            f8wins[w + 1] = convert_win(w + 1)
                    nc.gpsimd.collective_compute(
                        "ReduceScatter", mybir.AluOpType.add,
                        replica_groups=[list(range(NCORES))],
                        ins=[rs_in[w].opt()], outs=[rs_out[w].opt()])
                    # post-RS: rank i holds summed layer i (d-major) for
                    # this token window; copy into the [D, B] output.
                    nc.gpsimd.dma_start(out[:, boff:boff + wb], rs_out[w][:])
                    boff += wb

    nc.compile()
    return nc


def _get_nc():
    global _COMPILED_NC
    if _COMPILED_NC is None:
        _COMPILED_NC = _build_nc()
    return _COMPILED_NC


def _make_in_maps(x, W_enc, b_enc, W_dec, b_dec):
    bf16 = ml_dtypes.bfloat16
    e4m3 = ml_dtypes.float8_e4m3
    x = np.asarray(x, dtype=np.float32)
    W_enc = np.asarray(W_enc, dtype=np.float32)
    b_enc = np.asarray(b_enc, dtype=np.float32)
    W_dec = np.asarray(W_dec, dtype=np.float32)

    HN = B // EH
    # x -> [L, HN, 128, DT, EH] with d = k*128 + p, b = h*EH + t
    xt = x.transpose(0, 2, 1).reshape(L, DT, 128, HN, EH)
    xt = np.ascontiguousarray(xt.transpose(0, 3, 2, 1, 4)).astype(bf16)
    in_maps = []
    for i in range(NCORES):
        sl = slice(i * FL, (i + 1) * FL)
        # W_enc slice -> [L, 128, DT, FL]
        we = W_enc[:, sl, :].transpose(0, 2, 1).reshape(L, DT, 128, FL)
        wenc_i = np.ascontiguousarray(we.transpose(0, 2, 1, 3)).astype(bf16)
        benc_i = np.ascontiguousarray(
            b_enc[:, sl].reshape(L, AF, 128).transpose(2, 0, 1)
            .reshape(128, L * AF)).astype(np.float32)
        # W_dec slice -> [L, L, 128, AF, D] with feature = a*128 + p
        wd_i = W_dec[:, sl, :, :]                                  # [L,FL,L,D]
        wd = wd_i.reshape(L, AF, 128, L, D)
        wdec_i = np.ascontiguousarray(wd.transpose(0, 3, 2, 1, 4)).astype(bf16)
        # fp8 slices for F8J target layers, scaled by SW:
        # [L, nF8, 128, AF2, 2, D]; feature = t2*256 + i8*128 + p
        w8 = wd_i[:, :, list(F8J), :]                              # [L,FL,n8,D]
        w8 = w8.reshape(L, AF2, 2, 128, len(F8J), D)
        w8 = w8.transpose(0, 4, 3, 1, 2, 5)          # [L,n8,128,AF2,2,D]
        wdec8_i = np.ascontiguousarray(
            np.clip(w8 * SW, -240, 240).astype(e4m3))
        in_maps.append({"xt": xt, "wenc": wenc_i, "benc": benc_i,
                        "wdec": wdec_i, "wdec8": wdec8_i})
    return in_maps


def run(x, W_enc, b_enc, W_dec, b_dec, trace=False):
    """Run the kernel; returns (output [L, B, D] fp32, BassKernelResults)."""
    from concourse import bass_utils

    nc = _get_nc()
    in_maps = _make_in_maps(x, W_enc, b_enc, W_dec, b_dec)
    res = bass_utils.run_bass_kernel_spmd(
        nc, in_maps, core_ids=list(range(NCORES)), trace=trace)
    # each rank returns its layer d-major [D, B]
    outs = np.stack([res.results[i]["out"].astype(np.float32).T
                     for i in range(NCORES)], axis=0)
    full = outs + np.asarray(b_dec, np.float32)[:, None, :]
    return np.ascontiguousarray(full), res


def kernel(x, W_enc, b_enc, W_dec, b_dec):
    out, _ = run(x, W_enc, b_enc, W_dec, b_dec)
    return out


# revision 17
# speedup vs baseline: 1.0085x; 1.0085x over previous
"""Cross-layer transcoder kernel for 8 TRN2 NeuronCores.

Sharding: d_transcoder (F=4096) is split 8 ways (512 features per core).
Each core encodes all tokens against its feature slice, computes partial
cross-layer reconstructions for every target layer, and a chunked
ReduceScatter sums the partials; rank i receives target layer i
(d-major [D, B] per core, L == n_cores == 8).  The host transposes each
rank's output and adds the decoder bias.

Decode dataflow: W_dec tiles are the stationary operand, feats stream
512-token windows -> psum [128 d, 512 tok] x 6 d-tiles (6 banks), all
streams 512 long so LDWEIGHTS always hides under the previous stream.

Compute dtype: bf16 operands with fp32 PSUM accumulation, except target
layers F8J which are decoded in fp8-e4m3 DoubleRow (2 contraction slots
per pass, 2x PE throughput).  fp8 operands are pre-scaled (feats x16 on
device via DVE, W_dec x256 on host) and the drain de-scales by 2^-12.

All HBM streams use host-pretransposed contiguous layouts (>=6KB per
partition line) so the DMA stays efficient while the collective runs.

Queue routing: weight/x streaming loads on the Sync queue; partial
stores on the Scalar queue (after the DVE drains); the collective and
post-RS output copies on the GpSimd queue.
"""

import numpy as np
import ml_dtypes

L, B, D, F = 8, 2048, 768, 4096
NCORES = 8
FL = F // NCORES          # 512 features per core
AF = FL // 128            # 4 f-tiles per core
AF2 = AF // 2             # 2 fp8 double-tiles per core
DT = D // 128              # 6 d-tiles
EH = 512                  # encode token chunk per x DMA
WINS = [512, 512, 512, 512]
assert sum(WINS) == B

F8J = (0, 6)              # target layers decoded in fp8 DoubleRow
SF = 16.0                 # feats fp8 scale
SW = 256.0                # W_dec fp8 scale
DESCALE = 1.0 / (SF * SW)

_COMPILED_NC = None


def _build_nc():
    import concourse.mybir as mybir
    import concourse.tile as tile
    from concourse import bacc

    dt = mybir.dt
    nc = bacc.Bacc("TRN2", target_bir_lowering=False, debug=False,
                   num_devices=NCORES)

    HN = B // EH
    xt = nc.dram_tensor("xt", [L, HN, 128, DT, EH], dt.bfloat16,
                        kind="ExternalInput").ap()
    wenc = nc.dram_tensor("wenc", [L, 128, DT, FL], dt.bfloat16,
                          kind="ExternalInput").ap()
    benc = nc.dram_tensor("benc", [128, L * AF], dt.float32,
                          kind="ExternalInput").ap()
    wdec = nc.dram_tensor("wdec", [L, L, 128, AF, D], dt.bfloat16,
                          kind="ExternalInput").ap()
    wdec8 = nc.dram_tensor("wdec8", [L, len(F8J), 128, AF2, 2, D],
                           dt.float8e4, kind="ExternalInput").ap()
    out = nc.dram_tensor("out", [D, B], dt.bfloat16, kind="ExternalOutput").ap()

    RELU = mybir.ActivationFunctionType.Relu
    DR = mybir.MatmulPerfMode.DoubleRow

    with tile.TileContext(nc) as tc:
        with (
            tc.tile_pool(name="consts", bufs=1) as consts,
            tc.tile_pool(name="featp", bufs=L * AF) as featp,
            tc.tile_pool(name="decp", bufs=8) as decp,
            tc.tile_pool(name="dec8p", bufs=2) as dec8p,
            tc.tile_pool(name="dram", bufs=1, space="DRAM") as dram,
        ):
            feats = [
                [featp.tile([128, B], dt.bfloat16, name=f"feat_{l}_{a}",
                            tag="feat", bufs=L * AF) for a in range(AF)]
                for l in range(L)
            ]

            rs_in = [dram.tile([L, D, wb], dt.bfloat16, name=f"rs_in_{w}",
                               tag=f"rsin{w}") for w, wb in enumerate(WINS)]
            rs_out = [dram.tile([D, wb], dt.bfloat16, name=f"rs_out_{w}",
                                tag=f"rsout{w}") for w, wb in enumerate(WINS)]

            def load_wd(w, j, l):
                t = decp.tile([128, AF, D], dt.bfloat16, tag="wd", bufs=8,
                              name=f"wd_{w}_{j}_{l}")
                nc.sync.dma_start(t[:], wdec[l, j])
                return t

            def load_wd8(w, j8, l):
                t = dec8p.tile([128, AF2, 2, D], dt.float8e4, tag="wd8",
                               bufs=2, name=f"wd8_{w}_{j8}_{l}")
                nc.sync.dma_start(t[:], wdec8[l, j8])
                return t

            prefetched = {}

            # ---- Phase E: encode all layers/tokens; feats stay in SBUF ----
            with (
                tc.tile_pool(name="encp", bufs=2) as encp,
                tc.tile_pool(name="pep", bufs=3, space="PSUM") as pep,
            ):
                benc_t = None
                for l in range(L):
                    wenc_t = encp.tile([128, DT, FL], dt.bfloat16,
                                       tag="wenc_t", bufs=2, name=f"wenc_{l}")
                    nc.sync.dma_start(wenc_t[:], wenc[l])
                    for h in range(HN):
                        xt_t = encp.tile([128, DT, EH], dt.bfloat16,
                                         tag="xt_t", bufs=2, name=f"xt_{l}_{h}")
                        nc.sync.dma_start(xt_t[:], xt[l, h])
                        if l == 0 and h == 0 and benc_t is None:
                            benc_t = consts.tile([128, L * AF], dt.float32,
                                                 tag="benc_t")
                            nc.sync.dma_start(benc_t[:], benc)
                        for a in range(AF):
                            ps = pep.tile([128, EH], dt.float32,
                                          tag="pe", bufs=3,
                                          name=f"pe_{l}_{h}_{a}")
                            for k in range(DT):
                                nc.tensor.matmul(
                                    ps[:],
                                    wenc_t[:, k, a * 128:(a + 1) * 128],
                                    xt_t[:, k, :],
                                    start=(k == 0), stop=(k == DT - 1))
                            boff = h * EH
                            nc.scalar.activation(
                                feats[l][a][:, boff:boff + EH], ps[:],
                                RELU,
                                bias=benc_t[:, l * AF + a:l * AF + a + 1])
                    if l == 1:
                        # prefetch first bf16 decode weights behind encode
                        prefetched[(0, 1, 0)] = load_wd(0, 1, 0)
                        prefetched[(0, 1, 1)] = load_wd(0, 1, 1)
                        prefetched[(0, 2, 0)] = load_wd(0, 2, 0)

            # ---- Phase D: cross-layer decode + chunked ReduceScatter ----
            with (
                tc.tile_pool(name="f8p",
                             bufs=(max(F8J) + 1) * AF2 if F8J else 1) as f8p,
                tc.tile_pool(name="outp", bufs=6) as outp,
                tc.tile_pool(name="pdp", bufs=8, space="PSUM") as pdp,
            ):
                def convert_win(w):
                    # fp8 feats for window w (DVE runs these while the PE
                    # is busy with the previous window's last layers)
                    wb = WINS[w]
                    boff = sum(WINS[:w])
                    lmax = max(F8J)
                    d = {}
                    for l in range(lmax + 1):
                        for t2 in range(AF2):
                            ft = f8p.tile([128, 2, 512], dt.float8e4,
                                          tag="f8", bufs=(lmax + 1) * AF2,
                                          name=f"f8_{w}_{l}_{t2}")
                            for i in range(2):
                                nc.scalar.activation(
                                    ft[:, i, 0:wb],
                                    feats[l][2 * t2 + i][:, boff:boff + wb],
                                    RELU, scale=SF)
                            d[(l, t2)] = ft
                    return d

                f8wins = [None] * len(WINS)
                if F8J:
                    f8wins[0] = convert_win(0)
                boff = 0
                for w, wb in enumerate(WINS):
                    f8win = f8wins[w]
                    for j in range(L):
                        p6 = [pdp.tile([128, 512], dt.float32, tag="p6",
                                       bufs=8, name=f"p6_{w}_{j}_{q}")
                              for q in range(DT)]
                        if j in F8J:
                            j8 = F8J.index(j)
                            for l in range(j + 1):
                                wd8 = load_wd8(w, j8, l)
                                st = (l == 0)
                                sp = (l == j)
                                for t2 in range(AF2):
                                    for q in range(DT):
                                        nc.tensor.matmul(
                                            p6[q][:, 0:wb],
                                            wd8[:, t2, :,
                                                q * 128:(q + 1) * 128],
                                            f8win[(l, t2)][:, :, 0:wb],
                                            start=(st and t2 == 0),
                                            stop=(sp and t2 == AF2 - 1),
                                            perf_mode=DR)
                        else:
                            for l in range(j + 1):
                                wd = prefetched.pop((w, j, l), None)
                                if wd is None:
                                    wd = load_wd(w, j, l)
                                st = (l == 0)
                                sp = (l == j)
                                for a in range(AF):
                                    for q in range(DT):
                                        nc.tensor.matmul(
                                            p6[q][:, 0:wb],
                                            wd[:, a, q * 128:(q + 1) * 128],
                                            feats[l][a][:, boff:boff + wb],
                                            start=(st and a == 0),
                                            stop=(sp and a == AF - 1))
                        dscale = DESCALE if j in F8J else 1.0
                        for q in range(DT):
                            ot = outp.tile([128, 512], dt.bfloat16, tag="ot",
                                           bufs=6, name=f"ot_{w}_{j}_{q}")
                            nc.vector.tensor_scalar_mul(
                                ot[:, 0:wb], p6[q][:, 0:wb], dscale)
                            nc.scalar.dma_start(
                                rs_in[w][j, q * 128:(q + 1) * 128, :],
                                ot[:, 0:wb])
                        if (F8J and j == max(F8J)
                                and w + 1 < len(WINS)):
                            f8wins[w + 1] = convert_win(w + 1)
                    nc.gpsimd.collective_compute(
                        "ReduceScatter", mybir.AluOpType.add,
                        replica_groups=[list(range(NCORES))],
                        ins=[rs_in[w].opt()], outs=[rs_out[w].opt()])
                    # post-RS: rank i holds summed layer i (d-major) for
                    # this token window; copy into the [D, B] output.
                    nc.gpsimd.dma_start(out[:, boff:boff + wb], rs_out[w][:])
                    boff += wb

    nc.compile()
    return nc


def _get_nc():
    global _COMPILED_NC
    if _COMPILED_NC is None:
        _COMPILED_NC = _build_nc()
    return _COMPILED_NC


def _make_in_maps(x, W_enc, b_enc, W_dec, b_dec):
    bf16 = ml_dtypes.bfloat16
    e4m3 = ml_dtypes.float8_e4m3
    x = np.asarray(x, dtype=np.float32)
    W_enc = np.asarray(W_enc, dtype=np.float32)
    b_enc = np.asarray(b_enc, dtype=np.float32)
    W_dec = np.asarray(W_dec, dtype=np.float32)

    HN = B // EH
    # x -> [L, HN, 128, DT, EH] with d = k*128 + p, b = h*EH + t
    xt = x.transpose(0, 2, 1).reshape(L, DT, 128, HN, EH)
    xt = np.ascontiguousarray(xt.transpose(0, 3, 2, 1, 4)).astype(bf16)
    in_maps = []
    for i in range(NCORES):
        sl = slice(i * FL, (i + 1) * FL)
        # W_enc slice -> [L, 128, DT, FL]
        we = W_enc[:, sl, :].transpose(0, 2, 1).reshape(L, DT, 128, FL)
        wenc_i = np.ascontiguousarray(we.transpose(0, 2, 1, 3)).astype(bf16)
        benc_i = np.ascontiguousarray(
            b_enc[:, sl].reshape(L, AF, 128).transpose(2, 0, 1)
            .reshape(128, L * AF)).astype(np.float32)
        # W_dec slice -> [L, L, 128, AF, D] with feature = a*128 + p
        wd_i = W_dec[:, sl, :, :]                                  # [L,FL,L,D]
        wd = wd_i.reshape(L, AF, 128, L, D)
        wdec_i = np.ascontiguousarray(wd.transpose(0, 3, 2, 1, 4)).astype(bf16)
        # fp8 slices for F8J target layers, scaled by SW:
        # [L, nF8, 128, AF2, 2, D]; feature = t2*256 + i8*128 + p
        w8 = wd_i[:, :, list(F8J), :]                              # [L,FL,n8,D]
        w8 = w8.reshape(L, AF2, 2, 128, len(F8J), D)
        w8 = w8.transpose(0, 4, 3, 1, 2, 5)          # [L,n8,128,AF2,2,D]
        wdec8_i = np.ascontiguousarray(
            np.clip(w8 * SW, -240, 240).astype(e4m3))
        in_maps.append({"xt": xt, "wenc": wenc_i, "benc": benc_i,
                        "wdec": wdec_i, "wdec8": wdec8_i})
    return in_maps


def run(x, W_enc, b_enc, W_dec, b_dec, trace=False):
    """Run the kernel; returns (output [L, B, D] fp32, BassKernelResults)."""
    from concourse import bass_utils

    nc = _get_nc()
    in_maps = _make_in_maps(x, W_enc, b_enc, W_dec, b_dec)
    res = bass_utils.run_bass_kernel_spmd(
        nc, in_maps, core_ids=list(range(NCORES)), trace=trace)
    # each rank returns its layer d-major [D, B]
    outs = np.stack([res.results[i]["out"].astype(np.float32).T
                     for i in range(NCORES)], axis=0)
    full = outs + np.asarray(b_dec, np.float32)[:, None, :]
    return np.ascontiguousarray(full), res


def kernel(x, W_enc, b_enc, W_dec, b_dec):
    out, _ = run(x, W_enc, b_enc, W_dec, b_dec)
    return out


# revision 19
# speedup vs baseline: 1.0136x; 1.0051x over previous
"""Cross-layer transcoder kernel for 8 TRN2 NeuronCores.

Sharding: d_transcoder (F=4096) is split 8 ways (512 features per core).
Each core encodes all tokens against its feature slice, computes partial
cross-layer reconstructions for every target layer, and a chunked
ReduceScatter sums the partials; rank i receives target layer i
(d-major [D, B] per core, L == n_cores == 8).  The host transposes each
rank's output and adds the decoder bias.

Decode dataflow: W_dec tiles are the stationary operand, feats stream
512-token windows -> psum [128 d, 512 tok] x 6 d-tiles (6 banks), all
streams 512 long so LDWEIGHTS always hides under the previous stream.

Compute dtype: bf16 operands with fp32 PSUM accumulation, except target
layers F8J which are decoded in fp8-e4m3 DoubleRow (2 contraction slots
per pass, 2x PE throughput).  fp8 operands are pre-scaled (feats x16 on
device via DVE, W_dec x256 on host) and the drain de-scales by 2^-12.

All HBM streams use host-pretransposed contiguous layouts (>=6KB per
partition line) so the DMA stays efficient while the collective runs.

Queue routing: weight/x streaming loads on the Sync queue; partial
stores on the Scalar queue (after the DVE drains); the collective and
post-RS output copies on the GpSimd queue.
"""

import numpy as np
import ml_dtypes

L, B, D, F = 8, 2048, 768, 4096
NCORES = 8
FL = F // NCORES          # 512 features per core
AF = FL // 128            # 4 f-tiles per core
AF2 = AF // 2             # 2 fp8 double-tiles per core
DT = D // 128              # 6 d-tiles
EH = 512                  # encode token chunk per x DMA
WINS = [512, 512, 512, 512]
assert sum(WINS) == B

F8J = (0, 6)              # target layers decoded in fp8 DoubleRow
SF = 16.0                 # feats fp8 scale
SW = 256.0                # W_dec fp8 scale
DESCALE = 1.0 / (SF * SW)

_COMPILED_NC = None


def _build_nc():
    import concourse.mybir as mybir
    import concourse.tile as tile
    from concourse import bacc

    dt = mybir.dt
    nc = bacc.Bacc("TRN2", target_bir_lowering=False, debug=False,
                   num_devices=NCORES)

    HN = B // EH
    xt = nc.dram_tensor("xt", [L, HN, 128, DT, EH], dt.bfloat16,
                        kind="ExternalInput").ap()
    wenc = nc.dram_tensor("wenc", [L, 128, DT, FL], dt.bfloat16,
                          kind="ExternalInput").ap()
    benc = nc.dram_tensor("benc", [128, L * AF], dt.float32,
                          kind="ExternalInput").ap()
    wdec = nc.dram_tensor("wdec", [L, L, 128, AF, D], dt.bfloat16,
                          kind="ExternalInput").ap()
    wdec8 = nc.dram_tensor("wdec8", [L, len(F8J), 128, AF2, 2, D],
                           dt.float8e4, kind="ExternalInput").ap()
    out = nc.dram_tensor("out", [D, B], dt.bfloat16, kind="ExternalOutput").ap()

    RELU = mybir.ActivationFunctionType.Relu
    DR = mybir.MatmulPerfMode.DoubleRow

    with tile.TileContext(nc) as tc:
        with (
            tc.tile_pool(name="consts", bufs=1) as consts,
            tc.tile_pool(name="featp", bufs=L * AF) as featp,
            tc.tile_pool(name="decp", bufs=8) as decp,
            tc.tile_pool(name="dec8p", bufs=2) as dec8p,
            tc.tile_pool(name="dram", bufs=1, space="DRAM") as dram,
        ):
            feats = [
                [featp.tile([128, B], dt.bfloat16, name=f"feat_{l}_{a}",
                            tag="feat", bufs=L * AF) for a in range(AF)]
                for l in range(L)
            ]

            rs_in = [dram.tile([L, D, wb], dt.bfloat16, name=f"rs_in_{w}",
                               tag=f"rsin{w}") for w, wb in enumerate(WINS)]
            rs_out = [dram.tile([D, wb], dt.bfloat16, name=f"rs_out_{w}",
                                tag=f"rsout{w}") for w, wb in enumerate(WINS)]

            def load_wd(w, j, l):
                t = decp.tile([128, AF, D], dt.bfloat16, tag="wd", bufs=8,
                              name=f"wd_{w}_{j}_{l}")
                nc.sync.dma_start(t[:], wdec[l, j])
                return t

            def load_wd8(w, j8, l):
                t = dec8p.tile([128, AF2, 2, D], dt.float8e4, tag="wd8",
                               bufs=2, name=f"wd8_{w}_{j8}_{l}")
                nc.sync.dma_start(t[:], wdec8[l, j8])
                return t

            prefetched = {}

            # ---- Phase E: encode all layers/tokens; feats stay in SBUF ----
            with (
                tc.tile_pool(name="encp", bufs=2) as encp,
                tc.tile_pool(name="pep", bufs=4, space="PSUM") as pep,
            ):
                benc_t = None
                for l in range(L):
                    wenc_t = encp.tile([128, DT, FL], dt.bfloat16,
                                       tag="wenc_t", bufs=2, name=f"wenc_{l}")
                    nc.sync.dma_start(wenc_t[:], wenc[l])
                    for h in range(HN):
                        xt_t = encp.tile([128, DT, EH], dt.bfloat16,
                                         tag="xt_t", bufs=2, name=f"xt_{l}_{h}")
                        nc.sync.dma_start(xt_t[:], xt[l, h])
                        if l == 0 and h == 0 and benc_t is None:
                            benc_t = consts.tile([128, L * AF], dt.float32,
                                                 tag="benc_t")
                            nc.sync.dma_start(benc_t[:], benc)
                        for a in range(AF):
                            ps = pep.tile([128, EH], dt.float32,
                                          tag="pe", bufs=4,
                                          name=f"pe_{l}_{h}_{a}")
                            for k in range(DT):
                                nc.tensor.matmul(
                                    ps[:],
                                    wenc_t[:, k, a * 128:(a + 1) * 128],
                                    xt_t[:, k, :],
                                    start=(k == 0), stop=(k == DT - 1))
                            boff = h * EH
                            nc.scalar.activation(
                                feats[l][a][:, boff:boff + EH], ps[:],
                                RELU,
                                bias=benc_t[:, l * AF + a:l * AF + a + 1])
                    if l == 1:
                        # prefetch first bf16 decode weights behind encode
                        prefetched[(0, 1, 0)] = load_wd(0, 1, 0)
                        prefetched[(0, 1, 1)] = load_wd(0, 1, 1)
                        prefetched[(0, 2, 0)] = load_wd(0, 2, 0)

            # ---- Phase D: cross-layer decode + chunked ReduceScatter ----
            with (
                tc.tile_pool(name="f8p",
                             bufs=(max(F8J) + 1) * AF2 if F8J else 1) as f8p,
                tc.tile_pool(name="outp", bufs=7) as outp,
                tc.tile_pool(name="pdp", bufs=8, space="PSUM") as pdp,
            ):
                def convert_win(w):
                    # fp8 feats for window w (DVE runs these while the PE
                    # is busy with the previous window's last layers)
                    wb = WINS[w]
                    boff = sum(WINS[:w])
                    lmax = max(F8J)
                    d = {}
                    for l in range(lmax + 1):
                        for t2 in range(AF2):
                            ft = f8p.tile([128, 2, 512], dt.float8e4,
                                          tag="f8", bufs=(lmax + 1) * AF2,
                                          name=f"f8_{w}_{l}_{t2}")
                            for i in range(2):
                                nc.scalar.activation(
                                    ft[:, i, 0:wb],
                                    feats[l][2 * t2 + i][:, boff:boff + wb],
                                    RELU, scale=SF)
                            d[(l, t2)] = ft
                    return d

                f8wins = [None] * len(WINS)
                if F8J:
                    f8wins[0] = convert_win(0)
                boff = 0
                for w, wb in enumerate(WINS):
                    f8win = f8wins[w]
                    for j in range(L):
                        p6 = [pdp.tile([128, 512], dt.float32, tag="p6",
                                       bufs=8, name=f"p6_{w}_{j}_{q}")
                              for q in range(DT)]
                        if j in F8J:
                            j8 = F8J.index(j)
                            for l in range(j + 1):
                                wd8 = load_wd8(w, j8, l)
                                st = (l == 0)
                                sp = (l == j)
                                for t2 in range(AF2):
                                    for q in range(DT):
                                        nc.tensor.matmul(
                                            p6[q][:, 0:wb],
                                            wd8[:, t2, :,
                                                q * 128:(q + 1) * 128],
                                            f8win[(l, t2)][:, :, 0:wb],
                                            start=(st and t2 == 0),
                                            stop=(sp and t2 == AF2 - 1),
                                            perf_mode=DR)
                        else:
                            for l in range(j + 1):
                                wd = prefetched.pop((w, j, l), None)
                                if wd is None:
                                    wd = load_wd(w, j, l)
                                st = (l == 0)
                                sp = (l == j)
                                for a in range(AF):
                                    for q in range(DT):
                                        nc.tensor.matmul(
                                            p6[q][:, 0:wb],
                                            wd[:, a, q * 128:(q + 1) * 128],
                                            feats[l][a][:, boff:boff + wb],
                                            start=(st and a == 0),
                                            stop=(sp and a == AF - 1))
                        dscale = DESCALE if j in F8J else 1.0
                        for q in range(DT):
                            ot = outp.tile([128, 512], dt.bfloat16, tag="ot",
                                           bufs=7, name=f"ot_{w}_{j}_{q}")
                            nc.vector.tensor_scalar_mul(
                                ot[:, 0:wb], p6[q][:, 0:wb], dscale)
                            nc.scalar.dma_start(
                                rs_in[w][j, q * 128:(q + 1) * 128, :],
                                ot[:, 0:wb])
                        if (F8J and j == max(F8J)
                                and w + 1 < len(WINS)):
                            f8wins[w + 1] = convert_win(w + 1)
                    nc.gpsimd.collective_compute(
                        "ReduceScatter", mybir.AluOpType.add,
                        replica_groups=[list(range(NCORES))],
                        ins=[rs_in[w].opt()], outs=[rs_out[w].opt()])
                    # post-RS: rank i holds summed layer i (d-major) for
                    # this token window; copy into the [D, B] output.
                    nc.gpsimd.dma_start(out[:, boff:boff + wb], rs_out[w][:])
                    boff += wb

    nc.compile()
    return nc


def _get_nc():
    global _COMPILED_NC
    if _COMPILED_NC is None:
        _COMPILED_NC = _build_nc()
    return _COMPILED_NC


def _make_in_maps(x, W_enc, b_enc, W_dec, b_dec):
    bf16 = ml_dtypes.bfloat16
    e4m3 = ml_dtypes.float8_e4m3
    x = np.asarray(x, dtype=np.float32)
    W_enc = np.asarray(W_enc, dtype=np.float32)
    b_enc = np.asarray(b_enc, dtype=np.float32)
    W_dec = np.asarray(W_dec, dtype=np.float32)

    HN = B // EH
    # x -> [L, HN, 128, DT, EH] with d = k*128 + p, b = h*EH + t
    xt = x.transpose(0, 2, 1).reshape(L, DT, 128, HN, EH)
    xt = np.ascontiguousarray(xt.transpose(0, 3, 2, 1, 4)).astype(bf16)
    in_maps = []
    for i in range(NCORES):
        sl = slice(i * FL, (i + 1) * FL)
        # W_enc slice -> [L, 128, DT, FL]
        we = W_enc[:, sl, :].transpose(0, 2, 1).reshape(L, DT, 128, FL)
        wenc_i = np.ascontiguousarray(we.transpose(0, 2, 1, 3)).astype(bf16)
        benc_i = np.ascontiguousarray(
            b_enc[:, sl].reshape(L, AF, 128).transpose(2, 0, 1)
            .reshape(128, L * AF)).astype(np.float32)
        # W_dec slice -> [L, L, 128, AF, D] with feature = a*128 + p
        wd_i = W_dec[:, sl, :, :]                                  # [L,FL,L,D]
        wd = wd_i.reshape(L, AF, 128, L, D)
        wdec_i = np.ascontiguousarray(wd.transpose(0, 3, 2, 1, 4)).astype(bf16)
        # fp8 slices for F8J target layers, scaled by SW:
        # [L, nF8, 128, AF2, 2, D]; feature = t2*256 + i8*128 + p
        w8 = wd_i[:, :, list(F8J), :]                              # [L,FL,n8,D]
        w8 = w8.reshape(L, AF2, 2, 128, len(F8J), D)
        w8 = w8.transpose(0, 4, 3, 1, 2, 5)          # [L,n8,128,AF2,2,D]
        wdec8_i = np.ascontiguousarray(
            np.clip(w8 * SW, -240, 240).astype(e4m3))
        in_maps.append({"xt": xt, "wenc": wenc_i, "benc": benc_i,
                        "wdec": wdec_i, "wdec8": wdec8_i})
    return in_maps


def run(x, W_enc, b_enc, W_dec, b_dec, trace=False):
    """Run the kernel; returns (output [L, B, D] fp32, BassKernelResults)."""
    from concourse import bass_utils

    nc = _get_nc()
    in_maps = _make_in_maps(x, W_enc, b_enc, W_dec, b_dec)
    res = bass_utils.run_bass_kernel_spmd(
        nc, in_maps, core_ids=list(range(NCORES)), trace=trace)
    # each rank returns its layer d-major [D, B]
    outs = np.stack([res.results[i]["out"].astype(np.float32).T
                     for i in range(NCORES)], axis=0)
    full = outs + np.asarray(b_dec, np.float32)[:, None, :]
    return np.ascontiguousarray(full), res


def kernel(x, W_enc, b_enc, W_dec, b_dec):
    out, _ = run(x, W_enc, b_enc, W_dec, b_dec)
    return out


# revision 21
# speedup vs baseline: 1.0180x; 1.0043x over previous
"""Cross-layer transcoder kernel for 8 TRN2 NeuronCores.

Sharding: d_transcoder (F=4096) is split 8 ways (512 features per core).
Each core encodes all tokens against its feature slice, computes partial
cross-layer reconstructions for every target layer, and a chunked
ReduceScatter sums the partials; rank i receives target layer i
(d-major [D, B] per core, L == n_cores == 8).  The host transposes each
rank's output and adds the decoder bias.

Decode dataflow: W_dec tiles are the stationary operand, feats stream
512-token windows -> psum [128 d, 512 tok] x 6 d-tiles (6 banks), all
streams 512 long so LDWEIGHTS always hides under the previous stream.

Compute dtype: bf16 operands with fp32 PSUM accumulation, except target
layers F8J which are decoded in fp8-e4m3 DoubleRow (2 contraction slots
per pass, 2x PE throughput).  fp8 operands are pre-scaled (feats x16 on
device via DVE, W_dec x256 on host) and the drain de-scales by 2^-12.

All HBM streams use host-pretransposed contiguous layouts (>=6KB per
partition line) so the DMA stays efficient while the collective runs.

Queue routing: weight/x streaming loads on the Sync queue; partial
stores on the Scalar queue (after the DVE drains); the collective and
post-RS output copies on the GpSimd queue.
"""

import numpy as np
import ml_dtypes

L, B, D, F = 8, 2048, 768, 4096
NCORES = 8
FL = F // NCORES          # 512 features per core
AF = FL // 128            # 4 f-tiles per core
AF2 = AF // 2             # 2 fp8 double-tiles per core
DT = D // 128              # 6 d-tiles
EH = 512                  # encode token chunk per x DMA
WINS = [512, 512, 512, 512]
assert sum(WINS) == B

F8J = (0, 6)              # target layers decoded in fp8 DoubleRow
SF = 16.0                 # feats fp8 scale
SW = 256.0                # W_dec fp8 scale
DESCALE = 1.0 / (SF * SW)

_COMPILED_NC = None


def _build_nc():
    import concourse.mybir as mybir
    import concourse.tile as tile
    from concourse import bacc

    dt = mybir.dt
    nc = bacc.Bacc("TRN2", target_bir_lowering=False, debug=False,
                   num_devices=NCORES)

    HN = B // EH
    xt = nc.dram_tensor("xt", [L, HN, 128, DT, EH], dt.bfloat16,
                        kind="ExternalInput").ap()
    wenc = nc.dram_tensor("wenc", [L, 128, DT, FL], dt.bfloat16,
                          kind="ExternalInput").ap()
    benc = nc.dram_tensor("benc", [128, L * AF], dt.float32,
                          kind="ExternalInput").ap()
    wdec = nc.dram_tensor("wdec", [L, L, 128, AF, D], dt.bfloat16,
                          kind="ExternalInput").ap()
    wdec8 = nc.dram_tensor("wdec8", [L, len(F8J), 128, AF2, 2, D],
                           dt.float8e4, kind="ExternalInput").ap()
    out = nc.dram_tensor("out", [D, B], dt.bfloat16, kind="ExternalOutput").ap()

    RELU = mybir.ActivationFunctionType.Relu
    DR = mybir.MatmulPerfMode.DoubleRow

    with tile.TileContext(nc) as tc:
        with (
            tc.tile_pool(name="consts", bufs=1) as consts,
            tc.tile_pool(name="featp", bufs=L * AF) as featp,
            tc.tile_pool(name="decp", bufs=8) as decp,
            tc.tile_pool(name="dec8p", bufs=2) as dec8p,
            tc.tile_pool(name="dram", bufs=1, space="DRAM") as dram,
        ):
            feats = [
                [featp.tile([128, B], dt.bfloat16, name=f"feat_{l}_{a}",
                            tag="feat", bufs=L * AF) for a in range(AF)]
                for l in range(L)
            ]

            rs_in = [dram.tile([L, D, wb], dt.bfloat16, name=f"rs_in_{w}",
                               tag=f"rsin{w}") for w, wb in enumerate(WINS)]
            rs_out = [dram.tile([D, wb], dt.bfloat16, name=f"rs_out_{w}",
                                tag=f"rsout{w}") for w, wb in enumerate(WINS)]

            def load_wd(w, j, l):
                t = decp.tile([128, AF, D], dt.bfloat16, tag="wd", bufs=8,
                              name=f"wd_{w}_{j}_{l}")
                nc.sync.dma_start(t[:], wdec[l, j])
                return t

            def load_wd8(w, j8, l):
                t = dec8p.tile([128, AF2, 2, D], dt.float8e4, tag="wd8",
                               bufs=2, name=f"wd8_{w}_{j8}_{l}")
                nc.sync.dma_start(t[:], wdec8[l, j8])
                return t

            prefetched = {}

            # ---- Phase E: encode all layers/tokens; feats stay in SBUF ----
            with (
                tc.tile_pool(name="encp", bufs=2) as encp,
                tc.tile_pool(name="pep", bufs=4, space="PSUM") as pep,
            ):
                benc_t = None
                for l in range(L):
                    wenc_t = encp.tile([128, DT, FL], dt.bfloat16,
                                       tag="wenc_t", bufs=2, name=f"wenc_{l}")
                    nc.sync.dma_start(wenc_t[:], wenc[l])
                    for h in range(HN):
                        xt_t = encp.tile([128, DT, EH], dt.bfloat16,
                                         tag="xt_t", bufs=2, name=f"xt_{l}_{h}")
                        nc.sync.dma_start(xt_t[:], xt[l, h])
                        if l == 0 and h == 0 and benc_t is None:
                            benc_t = consts.tile([128, L * AF], dt.float32,
                                                 tag="benc_t")
                            nc.sync.dma_start(benc_t[:], benc)
                        for a in range(AF):
                            ps = pep.tile([128, EH], dt.float32,
                                          tag="pe", bufs=4,
                                          name=f"pe_{l}_{h}_{a}")
                            for k in range(DT):
                                nc.tensor.matmul(
                                    ps[:],
                                    wenc_t[:, k, a * 128:(a + 1) * 128],
                                    xt_t[:, k, :],
                                    start=(k == 0), stop=(k == DT - 1))
                            boff = h * EH
                            nc.scalar.activation(
                                feats[l][a][:, boff:boff + EH], ps[:],
                                RELU,
                                bias=benc_t[:, l * AF + a:l * AF + a + 1])
                    if l == 1:
                        # prefetch first bf16 decode weights behind encode
                        prefetched[(0, 1, 0)] = load_wd(0, 1, 0)
                        prefetched[(0, 1, 1)] = load_wd(0, 1, 1)
                        prefetched[(0, 2, 0)] = load_wd(0, 2, 0)

            # ---- Phase D: cross-layer decode + chunked ReduceScatter ----
            with (
                tc.tile_pool(name="f8p",
                             bufs=(max(F8J) + 1) * AF2 if F8J else 1) as f8p,
                tc.tile_pool(name="outp", bufs=7) as outp,
                tc.tile_pool(name="pdp", bufs=8, space="PSUM") as pdp,
            ):
                def convert_win(w):
                    # fp8 feats for window w (DVE runs these while the PE
                    # is busy with the previous window's last layers)
                    wb = WINS[w]
                    boff = sum(WINS[:w])
                    lmax = max(F8J)
                    d = {}
                    for l in range(lmax + 1):
                        for t2 in range(AF2):
                            ft = f8p.tile([128, 2, 512], dt.float8e4,
                                          tag="f8", bufs=(lmax + 1) * AF2,
                                          name=f"f8_{w}_{l}_{t2}")
                            for i in range(2):
                                nc.scalar.activation(
                                    ft[:, i, 0:wb],
                                    feats[l][2 * t2 + i][:, boff:boff + wb],
                                    RELU, scale=SF)
                            d[(l, t2)] = ft
                    return d

                f8wins = [None] * len(WINS)
                if F8J:
                    f8wins[0] = convert_win(0)
                boff = 0
                for w, wb in enumerate(WINS):
                    f8win = f8wins[w]
                    for j in range(L):
                        p6 = [pdp.tile([128, 512], dt.float32, tag="p6",
                                       bufs=8, name=f"p6_{w}_{j}_{q}")
                              for q in range(DT)]
                        if j in F8J:
                            j8 = F8J.index(j)
                            for l in range(j + 1):
                                wd8 = load_wd8(w, j8, l)
                                st = (l == 0)
                                sp = (l == j)
                                for t2 in range(AF2):
                                    for q in range(DT):
                                        nc.tensor.matmul(
                                            p6[q][:, 0:wb],
                                            wd8[:, t2, :,
                                                q * 128:(q + 1) * 128],
                                            f8win[(l, t2)][:, :, 0:wb],
                                            start=(st and t2 == 0),
                                            stop=(sp and t2 == AF2 - 1),
                                            perf_mode=DR)
                        else:
                            for l in range(j + 1):
                                wd = prefetched.pop((w, j, l), None)
                                if wd is None:
                                    wd = load_wd(w, j, l)
                                st = (l == 0)
                                sp = (l == j)
                                for a in range(AF):
                                    for q in range(DT):
                                        nc.tensor.matmul(
                                            p6[q][:, 0:wb],
                                            wd[:, a, q * 128:(q + 1) * 128],
                                            feats[l][a][:, boff:boff + wb],
                                            start=(st and a == 0),
                                            stop=(sp and a == AF - 1))
                        dscale = DESCALE if j in F8J else 1.0
                        for q in range(DT):
                            ot = outp.tile([128, 512], dt.bfloat16, tag="ot",
                                           bufs=7, name=f"ot_{w}_{j}_{q}")
                            nc.vector.tensor_scalar_mul(
                                ot[:, 0:wb], p6[q][:, 0:wb], dscale)
                            nc.scalar.dma_start(
                                rs_in[w][j, q * 128:(q + 1) * 128, :],
                                ot[:, 0:wb])
                        if (F8J and j == max(F8J)
                                and w + 1 < len(WINS)):
                            f8wins[w + 1] = convert_win(w + 1)
                    nc.gpsimd.collective_compute(
                        "ReduceScatter", mybir.AluOpType.add,
                        replica_groups=[list(range(NCORES))],
                        ins=[rs_in[w].opt()], outs=[rs_out[w].opt()])
                    # post-RS: rank i holds summed layer i (d-major) for
                    # this token window; copy into the [D, B] output.
                    nc.gpsimd.dma_start(out[:, boff:boff + wb], rs_out[w][:])
                    boff += wb

    nc.compile()
    return nc


def _get_nc():
    global _COMPILED_NC
    if _COMPILED_NC is None:
        _COMPILED_NC = _build_nc()
    return _COMPILED_NC


def _make_in_maps(x, W_enc, b_enc, W_dec, b_dec):
    bf16 = ml_dtypes.bfloat16
    e4m3 = ml_dtypes.float8_e4m3
    x = np.asarray(x, dtype=np.float32)
    W_enc = np.asarray(W_enc, dtype=np.float32)
    b_enc = np.asarray(b_enc, dtype=np.float32)
    W_dec = np.asarray(W_dec, dtype=np.float32)

    HN = B // EH
    # x -> [L, HN, 128, DT, EH] with d = k*128 + p, b = h*EH + t
    xt = x.transpose(0, 2, 1).reshape(L, DT, 128, HN, EH)
    xt = np.ascontiguousarray(xt.transpose(0, 3, 2, 1, 4)).astype(bf16)
    in_maps = []
    for i in range(NCORES):
        sl = slice(i * FL, (i + 1) * FL)
        # W_enc slice -> [L, 128, DT, FL]
        we = W_enc[:, sl, :].transpose(0, 2, 1).reshape(L, DT, 128, FL)
        wenc_i = np.ascontiguousarray(we.transpose(0, 2, 1, 3)).astype(bf16)
        benc_i = np.ascontiguousarray(
            b_enc[:, sl].reshape(L, AF, 128).transpose(2, 0, 1)
            .reshape(128, L * AF)).astype(np.float32)
        # W_dec slice -> [L, L, 128, AF, D] with feature = a*128 + p
        wd_i = W_dec[:, sl, :, :]                                  # [L,FL,L,D]
        wd = wd_i.reshape(L, AF, 128, L, D)
        wdec_i = np.ascontiguousarray(wd.transpose(0, 3, 2, 1, 4)).astype(bf16)
        # fp8 slices for F8J target layers, scaled by SW:
        # [L, nF8, 128, AF2, 2, D]; feature = t2*256 + i8*128 + p
        w8 = wd_i[:, :, list(F8J), :]                              # [L,FL,n8,D]
        w8 = w8.reshape(L, AF2, 2, 128, len(F8J), D)
        w8 = w8.transpose(0, 4, 3, 1, 2, 5)          # [L,n8,128,AF2,2,D]
        wdec8_i = np.ascontiguousarray(
            np.clip(w8 * SW, -240, 240).astype(e4m3))
        in_maps.append({"xt": xt, "wenc": wenc_i, "benc": benc_i,
                        "wdec": wdec_i, "wdec8": wdec8_i})
    return in_maps


def run(x, W_enc, b_enc, W_dec, b_dec, trace=False):
    """Run the kernel; returns (output [L, B, D] fp32, BassKernelResults)."""
    from concourse import bass_utils

    nc = _get_nc()
    in_maps = _make_in_maps(x, W_enc, b_enc, W_dec, b_dec)
    res = bass_utils.run_bass_kernel_spmd(
        nc, in_maps, core_ids=list(range(NCORES)), trace=trace)
    # each rank returns its layer d-major [D, B]
    outs = np.stack([res.results[i]["out"].astype(np.float32).T
                     for i in range(NCORES)], axis=0)
    full = outs + np.asarray(b_dec, np.float32)[:, None, :]
    return np.ascontiguousarray(full), res


def kernel(x, W_enc, b_enc, W_dec, b_dec):
    out, _ = run(x, W_enc, b_enc, W_dec, b_dec)
    return out


# revision 22
# speedup vs baseline: 1.0272x; 1.0090x over previous
"""Cross-layer transcoder kernel for 8 TRN2 NeuronCores.

Sharding: d_transcoder (F=4096) is split 8 ways (512 features per core).
Each core encodes all tokens against its feature slice, computes partial
cross-layer reconstructions for every target layer, and a chunked
ReduceScatter sums the partials; rank i receives target layer i
(d-major [D, B] per core, L == n_cores == 8).  The host transposes each
rank's output and adds the decoder bias.

Decode dataflow: W_dec tiles are the stationary operand, feats stream
512-token windows -> psum [128 d, 512 tok] x 6 d-tiles (6 banks), all
streams 512 long so LDWEIGHTS always hides under the previous stream.

Compute dtype: bf16 operands with fp32 PSUM accumulation, except target
layers F8J which are decoded in fp8-e4m3 DoubleRow (2 contraction slots
per pass, 2x PE throughput).  fp8 operands are pre-scaled (feats x16 on
device via DVE, W_dec x256 on host) and the drain de-scales by 2^-12.

All HBM streams use host-pretransposed contiguous layouts (>=6KB per
partition line) so the DMA stays efficient while the collective runs.

Queue routing: weight/x streaming loads on the Sync queue; partial
stores on the Scalar queue (after the DVE drains); the collective and
post-RS output copies on the GpSimd queue.
"""

import numpy as np
import ml_dtypes

L, B, D, F = 8, 2048, 768, 4096
NCORES = 8
FL = F // NCORES          # 512 features per core
AF = FL // 128            # 4 f-tiles per core
AF2 = AF // 2             # 2 fp8 double-tiles per core
DT = D // 128              # 6 d-tiles
EH = 512                  # encode token chunk per x DMA
WINS = [512, 512, 512, 512]
assert sum(WINS) == B

F8J = (0, 6)              # target layers decoded fully in fp8 DoubleRow
F8PART = {(0, 1): 2}      # (l, j) -> wdec8 slot: extra fp8 pairs inside
                          # bf16 groups (partner bf16 weights are host-
                          # scaled by SF*SW so the psum scale is uniform)
F8JW = (0, 6, 1)          # wdec8 slot -> target layer
F8SCALED = frozenset(F8J) | {j for (_, j) in F8PART}
SF = 16.0                 # feats fp8 scale
SW = 256.0                # W_dec fp8 scale
DESCALE = 1.0 / (SF * SW)

_COMPILED_NC = None


def _build_nc():
    import concourse.mybir as mybir
    import concourse.tile as tile
    from concourse import bacc

    dt = mybir.dt
    nc = bacc.Bacc("TRN2", target_bir_lowering=False, debug=False,
                   num_devices=NCORES)

    HN = B // EH
    xt = nc.dram_tensor("xt", [L, HN, 128, DT, EH], dt.bfloat16,
                        kind="ExternalInput").ap()
    wenc = nc.dram_tensor("wenc", [L, 128, DT, FL], dt.bfloat16,
                          kind="ExternalInput").ap()
    benc = nc.dram_tensor("benc", [128, L * AF], dt.float32,
                          kind="ExternalInput").ap()
    wdec = nc.dram_tensor("wdec", [L, L, 128, AF, D], dt.bfloat16,
                          kind="ExternalInput").ap()
    wdec8 = nc.dram_tensor("wdec8", [L, len(F8JW), 128, AF2, 2, D],
                           dt.float8e4, kind="ExternalInput").ap()
    out = nc.dram_tensor("out", [D, B], dt.bfloat16, kind="ExternalOutput").ap()

    RELU = mybir.ActivationFunctionType.Relu
    DR = mybir.MatmulPerfMode.DoubleRow

    with tile.TileContext(nc) as tc:
        with (
            tc.tile_pool(name="consts", bufs=1) as consts,
            tc.tile_pool(name="featp", bufs=L * AF) as featp,
            tc.tile_pool(name="decp", bufs=8) as decp,
            tc.tile_pool(name="dec8p", bufs=2) as dec8p,
            tc.tile_pool(name="dram", bufs=1, space="DRAM") as dram,
        ):
            feats = [
                [featp.tile([128, B], dt.bfloat16, name=f"feat_{l}_{a}",
                            tag="feat", bufs=L * AF) for a in range(AF)]
                for l in range(L)
            ]

            rs_in = [dram.tile([L, D, wb], dt.bfloat16, name=f"rs_in_{w}",
                               tag=f"rsin{w}") for w, wb in enumerate(WINS)]
            rs_out = [dram.tile([D, wb], dt.bfloat16, name=f"rs_out_{w}",
                                tag=f"rsout{w}") for w, wb in enumerate(WINS)]

            def load_wd(w, j, l):
                t = decp.tile([128, AF, D], dt.bfloat16, tag="wd", bufs=8,
                              name=f"wd_{w}_{j}_{l}")
                nc.sync.dma_start(t[:], wdec[l, j])
                return t

            def load_wd8(w, j8, l):
                t = dec8p.tile([128, AF2, 2, D], dt.float8e4, tag="wd8",
                               bufs=2, name=f"wd8_{w}_{j8}_{l}")
                nc.sync.dma_start(t[:], wdec8[l, j8])
                return t

            prefetched = {}

            # ---- Phase E: encode all layers/tokens; feats stay in SBUF ----
            with (
                tc.tile_pool(name="encp", bufs=2) as encp,
                tc.tile_pool(name="pep", bufs=4, space="PSUM") as pep,
            ):
                benc_t = None
                for l in range(L):
                    wenc_t = encp.tile([128, DT, FL], dt.bfloat16,
                                       tag="wenc_t", bufs=2, name=f"wenc_{l}")
                    nc.sync.dma_start(wenc_t[:], wenc[l])
                    for h in range(HN):
                        xt_t = encp.tile([128, DT, EH], dt.bfloat16,
                                         tag="xt_t", bufs=2, name=f"xt_{l}_{h}")
                        nc.sync.dma_start(xt_t[:], xt[l, h])
                        if l == 0 and h == 0 and benc_t is None:
                            benc_t = consts.tile([128, L * AF], dt.float32,
                                                 tag="benc_t")
                            nc.sync.dma_start(benc_t[:], benc)
                        for a in range(AF):
                            ps = pep.tile([128, EH], dt.float32,
                                          tag="pe", bufs=4,
                                          name=f"pe_{l}_{h}_{a}")
                            for k in range(DT):
                                nc.tensor.matmul(
                                    ps[:],
                                    wenc_t[:, k, a * 128:(a + 1) * 128],
                                    xt_t[:, k, :],
                                    start=(k == 0), stop=(k == DT - 1))
                            boff = h * EH
                            nc.scalar.activation(
                                feats[l][a][:, boff:boff + EH], ps[:],
                                RELU,
                                bias=benc_t[:, l * AF + a:l * AF + a + 1])
                    if l == 1:
                        # prefetch first bf16 decode weights behind encode
                        prefetched[(0, 1, 1)] = load_wd(0, 1, 1)
                        prefetched[(0, 2, 0)] = load_wd(0, 2, 0)
                        prefetched[(0, 2, 1)] = load_wd(0, 2, 1)

            # ---- Phase D: cross-layer decode + chunked ReduceScatter ----
            with (
                tc.tile_pool(name="f8p",
                             bufs=(max(F8J) + 1) * AF2 if F8J else 1) as f8p,
                tc.tile_pool(name="outp", bufs=7) as outp,
                tc.tile_pool(name="pdp", bufs=8, space="PSUM") as pdp,
            ):
                def convert_win(w):
                    # fp8 feats for window w (DVE runs these while the PE
                    # is busy with the previous window's last layers)
                    wb = WINS[w]
                    boff = sum(WINS[:w])
                    lmax = max(F8J)
                    d = {}
                    for l in range(lmax + 1):
                        for t2 in range(AF2):
                            ft = f8p.tile([128, 2, 512], dt.float8e4,
                                          tag="f8", bufs=(lmax + 1) * AF2,
                                          name=f"f8_{w}_{l}_{t2}")
                            for i in range(2):
                                nc.scalar.activation(
                                    ft[:, i, 0:wb],
                                    feats[l][2 * t2 + i][:, boff:boff + wb],
                                    RELU, scale=SF)
                            d[(l, t2)] = ft
                    return d

                f8wins = [None] * len(WINS)
                if F8J:
                    f8wins[0] = convert_win(0)
                boff = 0
                for w, wb in enumerate(WINS):
                    f8win = f8wins[w]
                    for j in range(L):
                        p6 = [pdp.tile([128, 512], dt.float32, tag="p6",
                                       bufs=8, name=f"p6_{w}_{j}_{q}")
                              for q in range(DT)]
                        if j in F8J:
                            j8 = F8J.index(j)
                            for l in range(j + 1):
                                wd8 = load_wd8(w, j8, l)
                                st = (l == 0)
                                sp = (l == j)
                                for t2 in range(AF2):
                                    for q in range(DT):
                                        nc.tensor.matmul(
                                            p6[q][:, 0:wb],
                                            wd8[:, t2, :,
                                                q * 128:(q + 1) * 128],
                                            f8win[(l, t2)][:, :, 0:wb],
                                            start=(st and t2 == 0),
                                            stop=(sp and t2 == AF2 - 1),
                                            perf_mode=DR)
                        else:
                            for l in range(j + 1):
                                slot = F8PART.get((l, j))
                                if slot is not None:
                                    wd8 = load_wd8(w, slot, l)
                                    for t2 in range(AF2):
                                        for q in range(DT):
                                            nc.tensor.matmul(
                                                p6[q][:, 0:wb],
                                                wd8[:, t2, :,
                                                    q * 128:(q + 1) * 128],
                                                f8win[(l, t2)][:, :, 0:wb],
                                                start=(l == 0 and t2 == 0),
                                                stop=(l == j
                                                      and t2 == AF2 - 1),
                                                perf_mode=DR)
                                    continue
                                wd = prefetched.pop((w, j, l), None)
                                if wd is None:
                                    wd = load_wd(w, j, l)
                                st = (l == 0)
                                sp = (l == j)
                                for a in range(AF):
                                    for q in range(DT):
                                        nc.tensor.matmul(
                                            p6[q][:, 0:wb],
                                            wd[:, a, q * 128:(q + 1) * 128],
                                            feats[l][a][:, boff:boff + wb],
                                            start=(st and a == 0),
                                            stop=(sp and a == AF - 1))
                        dscale = DESCALE if j in F8SCALED else 1.0
                        for q in range(DT):
                            ot = outp.tile([128, 512], dt.bfloat16, tag="ot",
                                           bufs=7, name=f"ot_{w}_{j}_{q}")
                            nc.vector.tensor_scalar_mul(
                                ot[:, 0:wb], p6[q][:, 0:wb], dscale)
                            nc.scalar.dma_start(
                                rs_in[w][j, q * 128:(q + 1) * 128, :],
                                ot[:, 0:wb])
                        if (F8J and j == max(F8J)
                                and w + 1 < len(WINS)):
                            f8wins[w + 1] = convert_win(w + 1)
                    nc.gpsimd.collective_compute(
                        "ReduceScatter", mybir.AluOpType.add,
                        replica_groups=[list(range(NCORES))],
                        ins=[rs_in[w].opt()], outs=[rs_out[w].opt()])
                    # post-RS: rank i holds summed layer i (d-major) for
                    # this token window; copy into the [D, B] output.
                    nc.gpsimd.dma_start(out[:, boff:boff + wb], rs_out[w][:])
                    boff += wb

    nc.compile()
    return nc


def _get_nc():
    global _COMPILED_NC
    if _COMPILED_NC is None:
        _COMPILED_NC = _build_nc()
    return _COMPILED_NC


def _make_in_maps(x, W_enc, b_enc, W_dec, b_dec):
    bf16 = ml_dtypes.bfloat16
    e4m3 = ml_dtypes.float8_e4m3
    x = np.asarray(x, dtype=np.float32)
    W_enc = np.asarray(W_enc, dtype=np.float32)
    b_enc = np.asarray(b_enc, dtype=np.float32)
    W_dec = np.asarray(W_dec, dtype=np.float32)

    HN = B // EH
    # x -> [L, HN, 128, DT, EH] with d = k*128 + p, b = h*EH + t
    xt = x.transpose(0, 2, 1).reshape(L, DT, 128, HN, EH)
    xt = np.ascontiguousarray(xt.transpose(0, 3, 2, 1, 4)).astype(bf16)
    in_maps = []
    for i in range(NCORES):
        sl = slice(i * FL, (i + 1) * FL)
        # W_enc slice -> [L, 128, DT, FL]
        we = W_enc[:, sl, :].transpose(0, 2, 1).reshape(L, DT, 128, FL)
        wenc_i = np.ascontiguousarray(we.transpose(0, 2, 1, 3)).astype(bf16)
        benc_i = np.ascontiguousarray(
            b_enc[:, sl].reshape(L, AF, 128).transpose(2, 0, 1)
            .reshape(128, L * AF)).astype(np.float32)
        # W_dec slice -> [L, L, 128, AF, D] with feature = a*128 + p.
        # bf16 weights of partial-fp8 target layers are pre-scaled by
        # SF*SW (exact exponent shift) so their psum matches the fp8
        # pairs' scale and the whole group drains with one descale.
        wd_i = W_dec[:, sl, :, :]                                  # [L,FL,L,D]
        wd_s = wd_i.copy()
        for jj in F8SCALED:
            if jj not in F8J:
                wd_s[:, :, jj, :] *= SF * SW
        wd = wd_s.reshape(L, AF, 128, L, D)
        wdec_i = np.ascontiguousarray(wd.transpose(0, 3, 2, 1, 4)).astype(bf16)
        # fp8 slices for F8JW target layers, scaled by SW:
        # [L, nF8, 128, AF2, 2, D]; feature = t2*256 + i8*128 + p
        w8 = wd_i[:, :, list(F8JW), :]                             # [L,FL,n8,D]
        w8 = w8.reshape(L, AF2, 2, 128, len(F8JW), D)
        w8 = w8.transpose(0, 4, 3, 1, 2, 5)          # [L,n8,128,AF2,2,D]
        wdec8_i = np.ascontiguousarray(
            np.clip(w8 * SW, -240, 240).astype(e4m3))
        in_maps.append({"xt": xt, "wenc": wenc_i, "benc": benc_i,
                        "wdec": wdec_i, "wdec8": wdec8_i})
    return in_maps


def run(x, W_enc, b_enc, W_dec, b_dec, trace=False):
    """Run the kernel; returns (output [L, B, D] fp32, BassKernelResults)."""
    from concourse import bass_utils

    nc = _get_nc()
    in_maps = _make_in_maps(x, W_enc, b_enc, W_dec, b_dec)
    res = bass_utils.run_bass_kernel_spmd(
        nc, in_maps, core_ids=list(range(NCORES)), trace=trace)
    # each rank returns its layer d-major [D, B]
    outs = np.stack([res.results[i]["out"].astype(np.float32).T
                     for i in range(NCORES)], axis=0)
    full = outs + np.asarray(b_dec, np.float32)[:, None, :]
    return np.ascontiguousarray(full), res


def kernel(x, W_enc, b_enc, W_dec, b_dec):
    out, _ = run(x, W_enc, b_enc, W_dec, b_dec)
    return out
